# revision 10
# baseline (speedup 1.0000x reference)
"""GRU-D cell kernel for Trainium2 (8 NeuronCores, data-parallel over batch).

Strategy
--------
Data-parallel: batch (16384) is split 8 ways -> 2048 rows/core. All weights
replicated per core. Everything on-chip is computed in a *feature-major*
(transposed) layout so that matmul contractions (over features) have the
contraction dim on SBUF partitions with zero on-chip transposes:

  gamma:  G.T[e_out, b]  = Wg @ delta.T      (lhsT = Wg.T)
  gates:  S.T[gate, b]   = W.T-blocks @ [x_t; mask; h].T

Precision: mixed fp8(e4m3, perf_mode=DoubleRow: 2 k-tiles per PE pass)
and fp16, selectable per contraction *block* of each gate via FLAGS
(11 blocks: gx, gh, r_x, r_m, r_h, z_x, z_m, z_h, in_x, in_m, hn).
Error-tolerant blocks (whole r gate, mask blocks, gamma) run fp8; the
error-amplified paths (z and tanh inputs over x_t/h) stay fp16. All
weights are pre-scaled x4096 (exact for fp16; fills e4m3's normal range
for fp8) so mixed-precision PSUM groups unscale with one activation
scale=1/4096. fp8 extras: error-feedback weight rounding along k (kills
the DC quantization error), delta streamed centered (+-0.5) with the DC
term folded exactly (using quantized weights) into the bias, binary mask
streamed raw (exact in fp8). On-chip intermediates that multiply large
values (r, z, h_n+b, n) are fp32; x_t/h are fp16.

The batch is processed in 4 chunks of 512 columns; each chunk runs
gamma -> prologue (x_t, h) -> gate matmuls -> epilogue, and the Tile
scheduler overlaps chunk c+1's DMA/gamma with chunk c's gate matmuls.
Weight packs are re-streamed per chunk (DMA has ~2x headroom).
"""

import os
from contextlib import ExitStack

import numpy as np
import ml_dtypes

import concourse.bass as bass
import concourse.mybir as mybir
import concourse.tile as tile
from concourse import bacc
from concourse.bass import ds
from concourse.bass_utils import run_bass_kernel_spmd

F16 = mybir.dt.float16
FP8 = mybir.dt.float8e4
F32 = mybir.dt.float32
NPF16 = np.float16
NPF8 = ml_dtypes.float8_e4m3

P = 128
E = 1024           # input size == hidden size
B = 16384
NCORES = 8
BC = B // NCORES   # 2048 batch rows per core
NB = 512           # batch-chunk (matmul moving free dim)
KE = E // P        # 8  feature k-tiles per block
JT = E // P        # 8  gate-feature tiles

AF = mybir.ActivationFunctionType
DR = mybir.MatmulPerfMode.DoubleRow

WS = 4096.0        # weight pre-scale (|w|*WS <= 128: fp8 normal, fp16 exact)
IVS = 1.0 / WS

# Per-block precision: '1' = fp8 DoubleRow, '0' = fp16.
# Order: [gx, gh, r_x, r_m, r_h, z_x, z_m, z_h, in_x, in_m, hn]
_cfg = os.environ.get("GRUD_FP8", "11111010011")
(F_GX, F_GH, F_RX, F_RM, F_RH, F_ZX, F_ZM, F_ZH,
 F_INX, F_INM, F_HN) = (c == "1" for c in _cfg)

GATE_BLOCKS = {          # contraction blocks per gate, in k order
    "r": ["x", "m", "h"],
    "z": ["x", "m", "h"],
    "in": ["x", "m"],
    "hn": ["h"],
}
GATE_FLAGS = {
    "r": {"x": F_RX, "m": F_RM, "h": F_RH},
    "z": {"x": F_ZX, "m": F_ZM, "h": F_ZH},
    "in": {"x": F_INX, "m": F_INM},
    "hn": {"h": F_HN},
}


def _blocks8(g):
    return [b for b in GATE_BLOCKS[g] if GATE_FLAGS[g][b]]


def _blocks16(g):
    return [b for b in GATE_BLOCKS[g] if not GATE_FLAGS[g][b]]


# Stash of the most recent hardware run info (read by test.py).
LAST_EXEC_NS = None
LAST_RESULTS = None


def build_gru_d(bc=BC, nb=NB):
    """Build the per-core Bass program (identical on all cores)."""
    nch = bc // nb
    nc = bacc.Bacc("TRN2", target_bir_lowering=False)

    need_d8 = F_GX or F_GH
    need_d16 = (not F_GX) or (not F_GH)
    need_m8 = F_RM or F_ZM or F_INM
    need_m16 = (not F_RM) or (not F_ZM) or (not F_INM)
    need_xt8 = F_RX or F_ZX or F_INX
    need_xt16 = (not F_RX) or (not F_ZX) or (not F_INX)
    need_h8 = F_RH or F_ZH or F_HN

    def wdt(f):
        return FP8 if f else F16

    # -- DRAM parameters (per core) --
    dcl = nc.declare_dram_parameter
    dT8 = dcl("dT8", [E, bc], FP8, isOutput=False) if need_d8 else None
    dT16 = dcl("dT16", [E, bc], F16, isOutput=False) if need_d16 else None
    mT8 = dcl("mT8", [E, bc], FP8, isOutput=False) if need_m8 else None
    mT16 = dcl("mT16", [E, bc], F16, isOutput=False) if need_m16 else None
    # A = m*x+(1-m)*mu, D = (1-m)*(l-mu), hs stacked; x_t = A + dx*D
    xlmh = dcl("xlmh", [3, E, bc], F16, isOutput=False)
    wgx = dcl("wgx", [KE, P, KE, P], wdt(F_GX), isOutput=False)
    wgh = dcl("wgh", [KE, P, KE, P], wdt(F_GH), isOutput=False)
    wg8 = {}   # per-gate fp8 packs [JT, P, 8*nblocks8, P]
    wg16 = {}  # per-gate fp16 packs
    for g in GATE_BLOCKS:
        n8, n16 = len(_blocks8(g)), len(_blocks16(g))
        if n8:
            wg8[g] = dcl(f"w8_{g}", [JT, P, KE * n8, P], FP8, isOutput=False)
        if n16:
            wg16[g] = dcl(f"w16_{g}", [JT, P, KE * n16, P], F16, isOutput=False)
    # biases are host-pretransposed to [P, ntiles] so the DMA is contiguous
    gbn = dcl("gbn", [P, 2 * KE], F32, isOutput=False)  # -(bias + fp8 DC fold)
    brz = dcl("brz", [P, 2 * JT], F32, isOutput=False)
    bnn = dcl("bnn", [P, JT], F32, isOutput=False)
    bhn = dcl("bhn", [P, JT], F32, isOutput=False)
    outT = dcl("outT", [E, bc], F32, isOutput=True)

    def fm(t):  # feature-major DRAM view: [E, bc] -> [p, ktile, b]
        return t[:].rearrange("(k p) b -> p k b", p=P)

    with ExitStack() as ctx:
        tc = ctx.enter_context(tile.TileContext(nc))
        p_bias = ctx.enter_context(tc.tile_pool(name="bias", bufs=1))
        p_psum = ctx.enter_context(tc.tile_pool(name="psum", bufs=8, space="PSUM"))
        p_act = ctx.enter_context(tc.tile_pool(name="acts", bufs=2))
        p_pk = ctx.enter_context(tc.tile_pool(name="pack", bufs=1))
        p_wg = ctx.enter_context(tc.tile_pool(name="wgp", bufs=3))
        p_w3 = ctx.enter_context(tc.tile_pool(name="w3p", bufs=2))
        p_g = ctx.enter_context(tc.tile_pool(name="gp", bufs=18))
        p_tmp = ctx.enter_context(tc.tile_pool(name="tmp", bufs=6))
        p_gact = ctx.enter_context(tc.tile_pool(name="gact", bufs=2))
        p_ep = ctx.enter_context(tc.tile_pool(name="ep", bufs=6))
        p_out = ctx.enter_context(tc.tile_pool(name="outp", bufs=3))

        # biases -> SBUF, feature-on-partition layout [128, ntiles]
        gbn_sb = p_bias.tile([P, 2 * KE], F32)
        nc.scalar.dma_start(out=gbn_sb, in_=gbn[:])
        brz_sb = p_bias.tile([P, 2 * JT], F32)
        nc.scalar.dma_start(out=brz_sb, in_=brz[:])
        bnn_sb = p_bias.tile([P, JT], F32)
        nc.scalar.dma_start(out=bnn_sb, in_=bnn[:])
        bhn_sb = p_bias.tile([P, JT], F32)
        nc.scalar.dma_start(out=bhn_sb, in_=bhn[:])

        wg_pf = []  # gamma packs for the next chunk, prefetched before the
        # gates weight stream enters the SP DMA ring
        for c in range(nch):
            cs = ds(c * nb, nb)
            # ---- chunk loads (ACT ring; weight packs go on the SP ring) ----
            d8_c = d16_c = m8_c = m16_c = None
            if need_d8:
                d8_c = p_act.tile([P, KE, nb], FP8, tag="d8c", name="d8_c",
                                  bufs=1)
                nc.scalar.dma_start(out=d8_c, in_=fm(dT8)[:, :, cs])
            if need_d16:
                d16_c = p_act.tile([P, KE, nb], F16, tag="d16c", name="d16_c",
                                   bufs=1)
                nc.scalar.dma_start(out=d16_c, in_=fm(dT16)[:, :, cs])
            xl_c = p_pk.tile([P, 3, KE, nb], F16, tag="xlmh")
            nc.gpsimd.dma_start(
                out=xl_c, in_=xlmh[:].rearrange("t (k p) b -> p t k b", p=P)[:, :, :, cs]
            )
            if need_m8:
                m8_c = p_act.tile([P, KE, nb], FP8, tag="m8c", name="m8_c")
                nc.scalar.dma_start(out=m8_c, in_=fm(mT8)[:, :, cs])
            if need_m16:
                m16_c = p_act.tile([P, KE, nb], F16, tag="m16c", name="m16_c")
                nc.scalar.dma_start(out=m16_c, in_=fm(mT16)[:, :, cs])
            xt8_c = (p_act.tile([P, KE, nb], FP8, tag="xt8c", name="xt8_c")
                     if need_xt8 else None)
            xt16_c = (p_act.tile([P, KE, nb], F16, tag="xt16c", name="xt16_c")
                      if need_xt16 else None)
            h16_c = p_act.tile([P, KE, nb], F16, tag="h16c")
            h8_c = (p_act.tile([P, KE, nb], FP8, tag="h8c", name="h8_c")
                    if need_h8 else None)

            srcs = {("x", True): xt8_c, ("x", False): xt16_c,
                    ("m", True): m8_c, ("m", False): m16_c,
                    ("h", True): h8_c, ("h", False): h16_c}

            # ---- gamma (dx/dh = exp(-relu(Wg @ delta.T + gb))) with the
            # prologue interleaved so x_t/h DVE work hides under gamma MMs.
            # Chunk 0: keep prologue DVE *after* all mins, so the DVE stream
            # doesn't block on the still-in-flight xlmh DMA and stall PSUM
            # recycling (min ops feed the e_t/psum slot chain). ----
            for mi in range(2 * KE):
                is_gx = mi < KE
                fp8g = F_GX if is_gx else F_GH
                wsrc = wgx[mi] if is_gx else wgh[mi - KE]
                d_c = d8_c if fp8g else d16_c
                if is_gx and wg_pf:  # pack prefetched during previous gates
                    wg_t = wg_pf.pop(0)
                else:
                    tg = "wgx" if is_gx else "wgh"
                    wg_t = p_wg.tile([P, KE, P], wdt(fp8g), tag=tg,
                                     bufs=(8 if is_gx else 4))
                    nc.sync.dma_start(out=wg_t, in_=wsrc)
                ps = p_psum.tile([P, nb], F32, tag="ps")
                if fp8g:
                    for q in range(KE // 2):
                        nc.tensor.matmul(ps, wg_t[:, ds(2 * q, 2), :],
                                         d_c[:, ds(2 * q, 2), :],
                                         start=(q == 0), stop=(q == KE // 2 - 1),
                                         perf_mode=DR)
                else:
                    for k in range(KE):
                        nc.tensor.matmul(ps, wg_t[:, k, :], d_c[:, k, :],
                                         start=(k == 0), stop=(k == KE - 1))
                # exp(-(u+b)) then min(.,1) == exp(-relu(u+b))
                # dx stays fp16 (feeds x_t); dh in f32 (feeds h directly)
                e_t = p_tmp.tile([P, nb], F16 if is_gx else F32,
                                 tag=("etx" if is_gx else "eth"), bufs=2,
                                 name="e_t")
                nc.scalar.activation(e_t, ps, AF.Exp, scale=-IVS,
                                     bias=gbn_sb[:, ds(mi, 1)])
                g_t = p_g.tile([P, nb], F16 if is_gx else F32,
                               tag=("gx_t" if is_gx else "gh_t"), bufs=8,
                               name="g_t")
                nc.vector.tensor_scalar_min(g_t, e_t, 1.0)

                def emit_prologue(mi=mi, g_t=g_t):
                    if mi < KE:
                        j = mi  # x_t[j] = A[j] + dx[j]*D[j]
                        t1 = p_tmp.tile([P, nb], F16, tag="xtmp", name="t1", bufs=2)
                        nc.vector.tensor_mul(t1, g_t, xl_c[:, 1, j, :])
                        if need_xt16:
                            nc.vector.tensor_add(xt16_c[:, j, :], t1, xl_c[:, 0, j, :])
                            if need_xt8:
                                nc.scalar.copy(xt8_c[:, j, :], xt16_c[:, j, :])
                        else:
                            nc.vector.tensor_add(xt8_c[:, j, :], t1, xl_c[:, 0, j, :])
                    else:
                        j = mi - KE  # h[j] = dh[j] * hs[j]
                        nc.vector.tensor_mul(h16_c[:, j, :], g_t, xl_c[:, 2, j, :])
                        if need_h8:
                            nc.scalar.copy(h8_c[:, j, :], h16_c[:, j, :])

                emit_prologue()

            if c + 1 < nch:  # prefetch next chunk's first gamma packs ahead
                for mi in range(4):  # of the gates weight stream (ring order)
                    wg_t = p_wg.tile([P, KE, P], wdt(F_GX), tag="wgx", bufs=8,
                                     name="wg_pf")
                    nc.sync.dma_start(out=wg_t, in_=wgx[mi])
                    wg_pf.append(wg_t)

            # ---- gates ----
            def emit_gate(g, j, ps):
                """All MMs of gate g for feature tile j into PSUM ps."""
                b8, b16 = _blocks8(g), _blocks16(g)
                w8_t = w16_t = None
                if b8:
                    w8_t = p_w3.tile([P, KE * len(b8), P], FP8,
                                     tag=f"w8{g}", name="w8_t",
                                     bufs=2)
                    nc.gpsimd.dma_start(out=w8_t, in_=wg8[g][j])
                if b16:
                    w16_t = p_w3.tile([P, KE * len(b16), P], F16,
                                      tag=f"w16{g}", name="w16_t",
                                      bufs=2)
                    nc.gpsimd.dma_start(out=w16_t, in_=wg16[g][j])
                n8 = KE * len(b8)
                npair = n8 // 2
                n16 = KE * len(b16)
                first = True
                for q in range(npair):
                    src = srcs[(b8[2 * q // KE], True)]
                    nc.tensor.matmul(ps, w8_t[:, ds(2 * q, 2), :],
                                     src[:, ds((2 * q) % KE, 2), :],
                                     start=first, stop=(q == npair - 1 and n16 == 0),
                                     perf_mode=DR)
                    first = False
                for k in range(n16):
                    src = srcs[(b16[k // KE], False)]
                    nc.tensor.matmul(ps, w16_t[:, k, :], src[:, k % KE, :],
                                     start=first, stop=(k == n16 - 1))
                    first = False

            # group order r, hn, in, z: the j-tail after the last MM group is
            # just sigmoid(z) + 2 DVE ops; tanh path overlaps the z matmuls
            for j in range(JT):
                ps = p_psum.tile([P, nb], F32, tag="ps")
                emit_gate("r", j, ps)
                r_t = p_gact.tile([P, nb], F32, tag="rt")
                nc.scalar.activation(r_t, ps, AF.Sigmoid, scale=IVS,
                                     bias=brz_sb[:, ds(j, 1)])

                ps = p_psum.tile([P, nb], F32, tag="ps")
                emit_gate("hn", j, ps)
                hnb_t = p_gact.tile([P, nb], F32, tag="hnbt")
                nc.scalar.activation(hnb_t, ps, AF.Identity, scale=IVS,
                                     bias=bhn_sb[:, ds(j, 1)])

                ps_in = p_psum.tile([P, nb], F32, tag="ps", name="ps_in")
                emit_gate("in", j, ps_in)
                # n = tanh(i_n + bnn + r*(h_n + bhn));  out = n + z*(h - n)
                i_t = p_ep.tile([P, nb], F32, tag="eptmp")
                nc.scalar.activation(i_t, ps_in, AF.Identity, scale=IVS,
                                     bias=bnn_sb[:, ds(j, 1)])
                t_m = p_ep.tile([P, nb], F32, tag="eptmp")
                nc.vector.tensor_mul(t_m, r_t, hnb_t)
                u_t = p_ep.tile([P, nb], F32, tag="eptmp")
                nc.vector.tensor_add(u_t, t_m, i_t)
                n_t = p_ep.tile([P, nb], F32, tag="eptmp")
                nc.scalar.activation(n_t, u_t, AF.Tanh)
                hm_t = p_ep.tile([P, nb], F32, tag="eptmp")
                nc.vector.tensor_sub(hm_t, h16_c[:, j, :], n_t)

                ps = p_psum.tile([P, nb], F32, tag="ps")
                emit_gate("z", j, ps)
                z_t = p_gact.tile([P, nb], F32, tag="zt")
                nc.scalar.activation(z_t, ps, AF.Sigmoid, scale=IVS,
                                     bias=brz_sb[:, ds(JT + j, 1)])
                zm_t = p_ep.tile([P, nb], F32, tag="eptmp")
                nc.vector.tensor_mul(zm_t, z_t, hm_t)
                o_t = p_out.tile([P, nb], F32, tag="ot")
                nc.vector.tensor_add(o_t, n_t, zm_t)
                nc.scalar.dma_start(out=outT[ds(j * P, P), cs], in_=o_t)
    nc.compile()
    return nc


def _q8_ef(w):
    """Error-feedback fp8 quantization along k (axis 0), in WS-scaled space.

    Returns (raw e4m3 [K, M], dequantized f32 [K, M] in WS space). The
    running residual keeps partial sums of (Wq - W) at ~one ulp, so the DC
    component of weight quantization error cancels against any constant
    input component.
    """
    ws = w.astype(np.float64) * WS
    q = np.empty(w.shape, NPF8)
    r = np.zeros(w.shape[1], np.float64)
    for k in range(w.shape[0]):
        t = ws[k] + r
        q[k] = np.clip(t, -240.0, 240.0).astype(NPF8)
        r = t - q[k].astype(np.float64)
    return q


def _pack(arr):
    """[K, M] -> [M//P, P, K//P, P] (value = arr[k*P+p, m*P+c])."""
    K, M = arr.shape
    return np.ascontiguousarray(
        arr.reshape(K // P, P, M // P, P).transpose(2, 1, 0, 3)
    )


def prep_shared(inputs):
    """Weights/biases shared by all cores, packed for the kernel."""
    gxw, gxb = inputs["gx_w"], inputs["gx_b"]
    ghw, ghb = inputs["gh_w"], inputs["gh_b"]
    wih, whh = inputs["w_ih"], inputs["w_hh"]
    bih, bhh = inputs["b_ih"], inputs["b_hh"]

    Wfull = np.concatenate([wih, whh], axis=0)          # [3E, 3E]
    # k-blocks of the gates contraction
    kblk = {"x": slice(0, E), "m": slice(E, 2 * E), "h": slice(2 * E, 3 * E)}
    mcol = {"r": slice(0, E), "z": slice(E, 2 * E), "in": slice(2 * E, 3 * E),
            "hn": slice(2 * E, 3 * E)}

    shared = {}

    def gamma_pack(wT, b, fp8, name):
        # wT = W.T [E(k), E(m)]; returns pack + bias (negated, with DC fold)
        if fp8:
            q = _q8_ef(wT)
            fold = 0.5 * (q.astype(np.float64) / WS).sum(axis=0)
            shared[name] = _pack(q)
            return -(b.astype(np.float64) + fold).astype(np.float32)
        shared[name] = _pack((wT.astype(np.float64) * WS).astype(NPF16))
        return -b.astype(np.float32)

    gbx = gamma_pack(np.ascontiguousarray(gxw.T), gxb, F_GX, "wgx")
    gbh = gamma_pack(np.ascontiguousarray(ghw.T), ghb, F_GH, "wgh")
    shared["gbn"] = np.concatenate([gbx, gbh])

    for g in GATE_BLOCKS:
        b8, b16 = _blocks8(g), _blocks16(g)
        if b8:
            w = np.concatenate(
                [_q8_ef(Wfull[kblk[b], mcol[g]]) for b in b8], axis=0)
            shared[f"w8_{g}"] = _pack(w)
        if b16:
            w = np.concatenate(
                [(Wfull[kblk[b], mcol[g]].astype(np.float64) * WS).astype(NPF16)
                 for b in b16], axis=0)
            shared[f"w16_{g}"] = _pack(w)

    def tp(v):  # [n*P] -> [P, n] feature-on-partition
        return np.ascontiguousarray(v.astype(np.float32).reshape(-1, P).T)

    shared["gbn"] = tp(shared["gbn"])
    shared["brz"] = tp((bih + bhh)[: 2 * E])
    shared["bnn"] = tp(bih[2 * E:])
    shared["bhn"] = tp(bhh[2 * E:])
    return shared


def prep_core(inputs, rows, shared):
    """Per-core input map: transposed activations + shared weights."""
    msk = inputs["x_mask"][rows]
    x = inputs["x"][rows]
    mu = inputs["x_mean"][rows]
    xl = inputs["x_last_observed"][rows]
    A = msk * x + (1.0 - msk) * mu
    D = (1.0 - msk) * (xl - mu)
    m = {
        "xlmh": np.stack([
            A.T.astype(NPF16),
            D.T.astype(NPF16),
            inputs["hs"][rows].T.astype(NPF16),
        ]),
    }
    dT = inputs["delta"][rows].T
    mT = msk.T
    if F_GX or F_GH:
        m["dT8"] = (dT - np.float32(0.5)).astype(NPF8)
    if (not F_GX) or (not F_GH):
        m["dT16"] = dT.astype(NPF16)
    if F_RM or F_ZM or F_INM:
        m["mT8"] = mT.astype(NPF8)      # 0/1: exact in fp8
    if (not F_RM) or (not F_ZM) or (not F_INM):
        m["mT16"] = mT.astype(NPF16)
    m.update(shared)
    return m


def kernel(**inputs):
    global LAST_EXEC_NS, LAST_RESULTS
    inputs = {k: np.asarray(v) for k, v in inputs.items()}
    nc = build_gru_d(BC, NB)
    shared = prep_shared(inputs)
    in_maps = [
        prep_core(inputs, slice(i * BC, (i + 1) * BC), shared) for i in range(NCORES)
    ]
    trace = bool(os.environ.get("GRUD_TRACE"))
    res = run_bass_kernel_spmd(nc, in_maps, list(range(NCORES)), trace=trace)
    LAST_RESULTS = res
    LAST_EXEC_NS = res.exec_time_ns
    out = np.empty((B, E), np.float32)
    for i in range(NCORES):
        out[i * BC : (i + 1) * BC] = res.results[i]["outT"].T
    return out


# revision 11
# speedup vs baseline: 1.0195x; 1.0195x over previous
"""GRU-D cell kernel for Trainium2 (8 NeuronCores, data-parallel over batch).

Strategy
--------
Data-parallel: batch (16384) is split 8 ways -> 2048 rows/core. All weights
replicated per core. Everything on-chip is computed in a *feature-major*
(transposed) layout so that matmul contractions (over features) have the
contraction dim on SBUF partitions with zero on-chip transposes:

  gamma:  G.T[e_out, b]  = Wg @ delta.T      (lhsT = Wg.T)
  gates:  S.T[gate, b]   = W.T-blocks @ [x_t; mask; h].T

Precision: mixed fp8(e4m3, perf_mode=DoubleRow: 2 k-tiles per PE pass)
and fp16, selectable per contraction *block* of each gate via FLAGS
(11 blocks: gx, gh, r_x, r_m, r_h, z_x, z_m, z_h, in_x, in_m, hn).
Error-tolerant blocks (whole r gate, mask blocks, gamma) run fp8; the
error-amplified paths (z and tanh inputs over x_t/h) stay fp16. All
weights are pre-scaled x4096 (exact for fp16; fills e4m3's normal range
for fp8) so mixed-precision PSUM groups unscale with one activation
scale=1/4096. fp8 extras: error-feedback weight rounding along k (kills
the DC quantization error), delta streamed centered (+-0.5) with the DC
term folded exactly (using quantized weights) into the bias, binary mask
streamed raw (exact in fp8). On-chip intermediates that multiply large
values (r, z, h_n+b, n) are fp32; x_t/h are fp16.

The batch is processed in 4 chunks of 512 columns; each chunk runs
gamma -> prologue (x_t, h) -> gate matmuls -> epilogue, and the Tile
scheduler overlaps chunk c+1's DMA/gamma with chunk c's gate matmuls.
Weight packs are re-streamed per chunk (DMA has ~2x headroom).
"""

import os
from contextlib import ExitStack

import numpy as np
import ml_dtypes

import concourse.bass as bass
import concourse.mybir as mybir
import concourse.tile as tile
from concourse import bacc
from concourse.bass import ds
from concourse.bass_utils import run_bass_kernel_spmd

F16 = mybir.dt.float16
FP8 = mybir.dt.float8e4
F32 = mybir.dt.float32
NPF16 = np.float16
NPF8 = ml_dtypes.float8_e4m3

P = 128
E = 1024           # input size == hidden size
B = 16384
NCORES = 8
BC = B // NCORES   # 2048 batch rows per core
NB = 512           # batch-chunk (matmul moving free dim)
KE = E // P        # 8  feature k-tiles per block
JT = E // P        # 8  gate-feature tiles

AF = mybir.ActivationFunctionType
DR = mybir.MatmulPerfMode.DoubleRow

WS = 4096.0        # weight pre-scale (|w|*WS <= 128: fp8 normal, fp16 exact)
IVS = 1.0 / WS

# Per-block precision: '1' = fp8 DoubleRow, '0' = fp16.
# Order: [gx, gh, r_x, r_m, r_h, z_x, z_m, z_h, in_x, in_m, hn]
_cfg = os.environ.get("GRUD_FP8", "11111010011")
(F_GX, F_GH, F_RX, F_RM, F_RH, F_ZX, F_ZM, F_ZH,
 F_INX, F_INM, F_HN) = (c == "1" for c in _cfg)

GATE_BLOCKS = {          # contraction blocks per gate, in k order
    "r": ["x", "m", "h"],
    "z": ["x", "m", "h"],
    "in": ["x", "m"],
    "hn": ["h"],
}
GATE_FLAGS = {
    "r": {"x": F_RX, "m": F_RM, "h": F_RH},
    "z": {"x": F_ZX, "m": F_ZM, "h": F_ZH},
    "in": {"x": F_INX, "m": F_INM},
    "hn": {"h": F_HN},
}


def _blocks8(g):
    return [b for b in GATE_BLOCKS[g] if GATE_FLAGS[g][b]]


def _blocks16(g):
    return [b for b in GATE_BLOCKS[g] if not GATE_FLAGS[g][b]]


# Stash of the most recent hardware run info (read by test.py).
LAST_EXEC_NS = None
LAST_RESULTS = None


def build_gru_d(bc=BC, nb=NB):
    """Build the per-core Bass program (identical on all cores)."""
    nch = bc // nb
    nc = bacc.Bacc("TRN2", target_bir_lowering=False)

    need_d8 = F_GX or F_GH
    need_d16 = (not F_GX) or (not F_GH)
    need_m8 = F_RM or F_ZM or F_INM
    need_m16 = (not F_RM) or (not F_ZM) or (not F_INM)
    need_xt8 = F_RX or F_ZX or F_INX
    need_xt16 = (not F_RX) or (not F_ZX) or (not F_INX)
    need_h8 = F_RH or F_ZH or F_HN

    def wdt(f):
        return FP8 if f else F16

    # -- DRAM parameters (per core) --
    dcl = nc.declare_dram_parameter
    # activation streams are host-packed chunk-major [P, nch, (t,) KE, nb]
    # so each per-chunk DMA is one long contiguous run per partition
    nch_ = bc // nb
    dT8 = dcl("dT8", [P, nch_, KE, nb], FP8, isOutput=False) if need_d8 else None
    dT16 = dcl("dT16", [P, nch_, KE, nb], F16, isOutput=False) if need_d16 else None
    mT8 = dcl("mT8", [P, nch_, KE, nb], FP8, isOutput=False) if need_m8 else None
    mT16 = dcl("mT16", [P, nch_, KE, nb], F16, isOutput=False) if need_m16 else None
    # A = m*x+(1-m)*mu, D = (1-m)*(l-mu), hs stacked; x_t = A + dx*D
    xlmh = dcl("xlmh", [P, nch_, 3, KE, nb], F16, isOutput=False)
    wgx = dcl("wgx", [KE, P, KE, P], wdt(F_GX), isOutput=False)
    wgh = dcl("wgh", [KE, P, KE, P], wdt(F_GH), isOutput=False)
    wg8 = {}   # per-gate fp8 packs [JT, P, 8*nblocks8, P]
    wg16 = {}  # per-gate fp16 packs
    for g in GATE_BLOCKS:
        n8, n16 = len(_blocks8(g)), len(_blocks16(g))
        if n8:
            wg8[g] = dcl(f"w8_{g}", [JT, P, KE * n8, P], FP8, isOutput=False)
        if n16:
            wg16[g] = dcl(f"w16_{g}", [JT, P, KE * n16, P], F16, isOutput=False)
    # biases are host-pretransposed to [P, ntiles] so the DMA is contiguous
    gbn = dcl("gbn", [P, 2 * KE], F32, isOutput=False)  # -(bias + fp8 DC fold)
    brz = dcl("brz", [P, 2 * JT], F32, isOutput=False)
    bnn = dcl("bnn", [P, JT], F32, isOutput=False)
    bhn = dcl("bhn", [P, JT], F32, isOutput=False)
    outT = dcl("outT", [E, bc], F32, isOutput=True)

    with ExitStack() as ctx:
        tc = ctx.enter_context(tile.TileContext(nc))
        p_bias = ctx.enter_context(tc.tile_pool(name="bias", bufs=1))
        p_psum = ctx.enter_context(tc.tile_pool(name="psum", bufs=8, space="PSUM"))
        p_act = ctx.enter_context(tc.tile_pool(name="acts", bufs=2))
        p_pk = ctx.enter_context(tc.tile_pool(name="pack", bufs=1))
        p_wg = ctx.enter_context(tc.tile_pool(name="wgp", bufs=3))
        p_w3 = ctx.enter_context(tc.tile_pool(name="w3p", bufs=2))
        p_g = ctx.enter_context(tc.tile_pool(name="gp", bufs=18))
        p_tmp = ctx.enter_context(tc.tile_pool(name="tmp", bufs=6))
        p_gact = ctx.enter_context(tc.tile_pool(name="gact", bufs=2))
        p_ep = ctx.enter_context(tc.tile_pool(name="ep", bufs=6))
        p_out = ctx.enter_context(tc.tile_pool(name="outp", bufs=3))

        # biases -> SBUF, feature-on-partition layout [128, ntiles]
        gbn_sb = p_bias.tile([P, 2 * KE], F32)
        nc.scalar.dma_start(out=gbn_sb, in_=gbn[:])
        brz_sb = p_bias.tile([P, 2 * JT], F32)
        nc.scalar.dma_start(out=brz_sb, in_=brz[:])
        bnn_sb = p_bias.tile([P, JT], F32)
        nc.scalar.dma_start(out=bnn_sb, in_=bnn[:])
        bhn_sb = p_bias.tile([P, JT], F32)
        nc.scalar.dma_start(out=bhn_sb, in_=bhn[:])

        wg_pf = []  # gamma packs for the next chunk, prefetched before the
        # gates weight stream enters the SP DMA ring
        for c in range(nch):
            cs = ds(c * nb, nb)
            # ---- chunk loads (ACT ring; weight packs go on the SP ring) ----
            d8_c = d16_c = m8_c = m16_c = None
            if need_d8:
                d8_c = p_act.tile([P, KE, nb], FP8, tag="d8c", name="d8_c",
                                  bufs=1)
                nc.scalar.dma_start(out=d8_c, in_=dT8[:, c])
            if need_d16:
                d16_c = p_act.tile([P, KE, nb], F16, tag="d16c", name="d16_c",
                                   bufs=1)
                nc.scalar.dma_start(out=d16_c, in_=dT16[:, c])
            xl_c = p_pk.tile([P, 3, KE, nb], F16, tag="xlmh")
            nc.gpsimd.dma_start(out=xl_c, in_=xlmh[:, c])
            if need_m8:
                m8_c = p_act.tile([P, KE, nb], FP8, tag="m8c", name="m8_c")
                nc.scalar.dma_start(out=m8_c, in_=mT8[:, c])
            if need_m16:
                m16_c = p_act.tile([P, KE, nb], F16, tag="m16c", name="m16_c")
                nc.scalar.dma_start(out=m16_c, in_=mT16[:, c])
            xt8_c = (p_act.tile([P, KE, nb], FP8, tag="xt8c", name="xt8_c")
                     if need_xt8 else None)
            xt16_c = (p_act.tile([P, KE, nb], F16, tag="xt16c", name="xt16_c")
                      if need_xt16 else None)
            h16_c = p_act.tile([P, KE, nb], F16, tag="h16c")
            h8_c = (p_act.tile([P, KE, nb], FP8, tag="h8c", name="h8_c")
                    if need_h8 else None)

            srcs = {("x", True): xt8_c, ("x", False): xt16_c,
                    ("m", True): m8_c, ("m", False): m16_c,
                    ("h", True): h8_c, ("h", False): h16_c}

            # ---- gamma (dx/dh = exp(-relu(Wg @ delta.T + gb))) with the
            # prologue interleaved so x_t/h DVE work hides under gamma MMs.
            # Chunk 0: keep prologue DVE *after* all mins, so the DVE stream
            # doesn't block on the still-in-flight xlmh DMA and stall PSUM
            # recycling (min ops feed the e_t/psum slot chain). ----
            for mi in range(2 * KE):
                is_gx = mi < KE
                fp8g = F_GX if is_gx else F_GH
                wsrc = wgx[mi] if is_gx else wgh[mi - KE]
                d_c = d8_c if fp8g else d16_c
                if is_gx and wg_pf:  # pack prefetched during previous gates
                    wg_t = wg_pf.pop(0)
                else:
                    tg = "wgx" if is_gx else "wgh"
                    wg_t = p_wg.tile([P, KE, P], wdt(fp8g), tag=tg,
                                     bufs=(8 if is_gx else 4))
                    nc.sync.dma_start(out=wg_t, in_=wsrc)
                ps = p_psum.tile([P, nb], F32, tag="ps")
                if fp8g:
                    for q in range(KE // 2):
                        nc.tensor.matmul(ps, wg_t[:, ds(2 * q, 2), :],
                                         d_c[:, ds(2 * q, 2), :],
                                         start=(q == 0), stop=(q == KE // 2 - 1),
                                         perf_mode=DR)
                else:
                    for k in range(KE):
                        nc.tensor.matmul(ps, wg_t[:, k, :], d_c[:, k, :],
                                         start=(k == 0), stop=(k == KE - 1))
                # exp(-(u+b)) then min(.,1) == exp(-relu(u+b))
                # dx stays fp16 (feeds x_t); dh in f32 (feeds h directly)
                e_t = p_tmp.tile([P, nb], F16 if is_gx else F32,
                                 tag=("etx" if is_gx else "eth"), bufs=2,
                                 name="e_t")
                nc.scalar.activation(e_t, ps, AF.Exp, scale=-IVS,
                                     bias=gbn_sb[:, ds(mi, 1)])
                g_t = p_g.tile([P, nb], F16 if is_gx else F32,
                               tag=("gx_t" if is_gx else "gh_t"), bufs=8,
                               name="g_t")
                nc.vector.tensor_scalar_min(g_t, e_t, 1.0)

                def emit_prologue(mi=mi, g_t=g_t):
                    if mi < KE:
                        j = mi  # x_t[j] = A[j] + dx[j]*D[j]
                        t1 = p_tmp.tile([P, nb], F16, tag="xtmp", name="t1", bufs=2)
                        nc.vector.tensor_mul(t1, g_t, xl_c[:, 1, j, :])
                        if need_xt16:
                            nc.vector.tensor_add(xt16_c[:, j, :], t1, xl_c[:, 0, j, :])
                            if need_xt8:
                                nc.scalar.copy(xt8_c[:, j, :], xt16_c[:, j, :])
                        else:
                            nc.vector.tensor_add(xt8_c[:, j, :], t1, xl_c[:, 0, j, :])
                    else:
                        j = mi - KE  # h[j] = dh[j] * hs[j]
                        nc.vector.tensor_mul(h16_c[:, j, :], g_t, xl_c[:, 2, j, :])
                        if need_h8:
                            nc.scalar.copy(h8_c[:, j, :], h16_c[:, j, :])

                emit_prologue()

            if c + 1 < nch:  # prefetch next chunk's first gamma packs ahead
                for mi in range(4):  # of the gates weight stream (ring order)
                    wg_t = p_wg.tile([P, KE, P], wdt(F_GX), tag="wgx", bufs=8,
                                     name="wg_pf")
                    nc.sync.dma_start(out=wg_t, in_=wgx[mi])
                    wg_pf.append(wg_t)

            # ---- gates ----
            def emit_gate(g, j, ps):
                """All MMs of gate g for feature tile j into PSUM ps."""
                b8, b16 = _blocks8(g), _blocks16(g)
                w8_t = w16_t = None
                if b8:
                    w8_t = p_w3.tile([P, KE * len(b8), P], FP8,
                                     tag=f"w8{g}", name="w8_t",
                                     bufs=2)
                    nc.gpsimd.dma_start(out=w8_t, in_=wg8[g][j])
                if b16:
                    w16_t = p_w3.tile([P, KE * len(b16), P], F16,
                                      tag=f"w16{g}", name="w16_t",
                                      bufs=2)
                    nc.gpsimd.dma_start(out=w16_t, in_=wg16[g][j])
                n8 = KE * len(b8)
                npair = n8 // 2
                n16 = KE * len(b16)
                first = True
                for q in range(npair):
                    src = srcs[(b8[2 * q // KE], True)]
                    nc.tensor.matmul(ps, w8_t[:, ds(2 * q, 2), :],
                                     src[:, ds((2 * q) % KE, 2), :],
                                     start=first, stop=(q == npair - 1 and n16 == 0),
                                     perf_mode=DR)
                    first = False
                for k in range(n16):
                    src = srcs[(b16[k // KE], False)]
                    nc.tensor.matmul(ps, w16_t[:, k, :], src[:, k % KE, :],
                                     start=first, stop=(k == n16 - 1))
                    first = False

            # group order r, hn, in, z: the j-tail after the last MM group is
            # just sigmoid(z) + 2 DVE ops; tanh path overlaps the z matmuls
            for j in range(JT):
                ps = p_psum.tile([P, nb], F32, tag="ps")
                emit_gate("r", j, ps)
                r_t = p_gact.tile([P, nb], F32, tag="rt")
                nc.scalar.activation(r_t, ps, AF.Sigmoid, scale=IVS,
                                     bias=brz_sb[:, ds(j, 1)])

                ps = p_psum.tile([P, nb], F32, tag="ps")
                emit_gate("hn", j, ps)
                hnb_t = p_gact.tile([P, nb], F32, tag="hnbt")
                nc.scalar.activation(hnb_t, ps, AF.Identity, scale=IVS,
                                     bias=bhn_sb[:, ds(j, 1)])

                ps_in = p_psum.tile([P, nb], F32, tag="ps", name="ps_in")
                emit_gate("in", j, ps_in)
                # n = tanh(i_n + bnn + r*(h_n + bhn));  out = n + z*(h - n)
                i_t = p_ep.tile([P, nb], F32, tag="eptmp")
                nc.scalar.activation(i_t, ps_in, AF.Identity, scale=IVS,
                                     bias=bnn_sb[:, ds(j, 1)])
                t_m = p_ep.tile([P, nb], F32, tag="eptmp")
                nc.vector.tensor_mul(t_m, r_t, hnb_t)
                u_t = p_ep.tile([P, nb], F32, tag="eptmp")
                nc.vector.tensor_add(u_t, t_m, i_t)
                n_t = p_ep.tile([P, nb], F32, tag="eptmp")
                nc.scalar.activation(n_t, u_t, AF.Tanh)
                hm_t = p_ep.tile([P, nb], F32, tag="eptmp")
                nc.vector.tensor_sub(hm_t, h16_c[:, j, :], n_t)

                ps = p_psum.tile([P, nb], F32, tag="ps")
                emit_gate("z", j, ps)
                z_t = p_gact.tile([P, nb], F32, tag="zt")
                nc.scalar.activation(z_t, ps, AF.Sigmoid, scale=IVS,
                                     bias=brz_sb[:, ds(JT + j, 1)])
                zm_t = p_ep.tile([P, nb], F32, tag="eptmp")
                nc.vector.tensor_mul(zm_t, z_t, hm_t)
                o_t = p_out.tile([P, nb], F32, tag="ot")
                nc.vector.tensor_add(o_t, n_t, zm_t)
                nc.scalar.dma_start(out=outT[ds(j * P, P), cs], in_=o_t)
    nc.compile()
    return nc


def _q8_ef(w):
    """Error-feedback fp8 quantization along k (axis 0), in WS-scaled space.

    Returns (raw e4m3 [K, M], dequantized f32 [K, M] in WS space). The
    running residual keeps partial sums of (Wq - W) at ~one ulp, so the DC
    component of weight quantization error cancels against any constant
    input component.
    """
    ws = w.astype(np.float64) * WS
    q = np.empty(w.shape, NPF8)
    r = np.zeros(w.shape[1], np.float64)
    for k in range(w.shape[0]):
        t = ws[k] + r
        q[k] = np.clip(t, -240.0, 240.0).astype(NPF8)
        r = t - q[k].astype(np.float64)
    return q


def _pack(arr):
    """[K, M] -> [M//P, P, K//P, P] (value = arr[k*P+p, m*P+c])."""
    K, M = arr.shape
    return np.ascontiguousarray(
        arr.reshape(K // P, P, M // P, P).transpose(2, 1, 0, 3)
    )


def prep_shared(inputs):
    """Weights/biases shared by all cores, packed for the kernel."""
    gxw, gxb = inputs["gx_w"], inputs["gx_b"]
    ghw, ghb = inputs["gh_w"], inputs["gh_b"]
    wih, whh = inputs["w_ih"], inputs["w_hh"]
    bih, bhh = inputs["b_ih"], inputs["b_hh"]

    Wfull = np.concatenate([wih, whh], axis=0)          # [3E, 3E]
    # k-blocks of the gates contraction
    kblk = {"x": slice(0, E), "m": slice(E, 2 * E), "h": slice(2 * E, 3 * E)}
    mcol = {"r": slice(0, E), "z": slice(E, 2 * E), "in": slice(2 * E, 3 * E),
            "hn": slice(2 * E, 3 * E)}

    shared = {}

    def gamma_pack(wT, b, fp8, name):
        # wT = W.T [E(k), E(m)]; returns pack + bias (negated, with DC fold)
        if fp8:
            q = _q8_ef(wT)
            fold = 0.5 * (q.astype(np.float64) / WS).sum(axis=0)
            shared[name] = _pack(q)
            return -(b.astype(np.float64) + fold).astype(np.float32)
        shared[name] = _pack((wT.astype(np.float64) * WS).astype(NPF16))
        return -b.astype(np.float32)

    gbx = gamma_pack(np.ascontiguousarray(gxw.T), gxb, F_GX, "wgx")
    gbh = gamma_pack(np.ascontiguousarray(ghw.T), ghb, F_GH, "wgh")
    shared["gbn"] = np.concatenate([gbx, gbh])

    for g in GATE_BLOCKS:
        b8, b16 = _blocks8(g), _blocks16(g)
        if b8:
            w = np.concatenate(
                [_q8_ef(Wfull[kblk[b], mcol[g]]) for b in b8], axis=0)
            shared[f"w8_{g}"] = _pack(w)
        if b16:
            w = np.concatenate(
                [(Wfull[kblk[b], mcol[g]].astype(np.float64) * WS).astype(NPF16)
                 for b in b16], axis=0)
            shared[f"w16_{g}"] = _pack(w)

    def tp(v):  # [n*P] -> [P, n] feature-on-partition
        return np.ascontiguousarray(v.astype(np.float32).reshape(-1, P).T)

    shared["gbn"] = tp(shared["gbn"])
    shared["brz"] = tp((bih + bhh)[: 2 * E])
    shared["bnn"] = tp(bih[2 * E:])
    shared["bhn"] = tp(bhh[2 * E:])
    return shared


def prep_core(inputs, rows, shared):
    """Per-core input map: transposed activations + shared weights."""
    msk = inputs["x_mask"][rows]
    x = inputs["x"][rows]
    mu = inputs["x_mean"][rows]
    xl = inputs["x_last_observed"][rows]
    A = msk * x + (1.0 - msk) * mu
    D = (1.0 - msk) * (xl - mu)
    bc = A.shape[0]
    nch = bc // NB

    def cm(a):  # [rows, E] -> chunk-major [P, nch, KE, nb]
        return np.ascontiguousarray(
            a.T.reshape(KE, P, nch, NB).transpose(1, 2, 0, 3))

    m = {
        "xlmh": np.ascontiguousarray(np.stack([
            cm(A.astype(NPF16)),
            cm(D.astype(NPF16)),
            cm(inputs["hs"][rows].astype(NPF16)),
        ], axis=2)),
    }
    if F_GX or F_GH:
        m["dT8"] = cm((inputs["delta"][rows] - np.float32(0.5)).astype(NPF8))
    if (not F_GX) or (not F_GH):
        m["dT16"] = cm(inputs["delta"][rows].astype(NPF16))
    if F_RM or F_ZM or F_INM:
        m["mT8"] = cm(msk.astype(NPF8))      # 0/1: exact in fp8
    if (not F_RM) or (not F_ZM) or (not F_INM):
        m["mT16"] = cm(msk.astype(NPF16))
    m.update(shared)
    return m


def kernel(**inputs):
    global LAST_EXEC_NS, LAST_RESULTS
    inputs = {k: np.asarray(v) for k, v in inputs.items()}
    nc = build_gru_d(BC, NB)
    shared = prep_shared(inputs)
    in_maps = [
        prep_core(inputs, slice(i * BC, (i + 1) * BC), shared) for i in range(NCORES)
    ]
    trace = bool(os.environ.get("GRUD_TRACE"))
    res = run_bass_kernel_spmd(nc, in_maps, list(range(NCORES)), trace=trace)
    LAST_RESULTS = res
    LAST_EXEC_NS = res.exec_time_ns
    out = np.empty((B, E), np.float32)
    for i in range(NCORES):
        out[i * BC : (i + 1) * BC] = res.results[i]["outT"].T
    return out
